# revision 1
# baseline (speedup 1.0000x reference)
"""VQ codebook-lookup kernel for Trainium2 (8 NeuronCores, data-parallel over batch).

e[b,t,:] = dictionary[argmin_n ||ze[b,t,:] - dictionary[n,:]||^2]

Per core: rows = 4 batches x 2048 = 8192, tiled 64 x 128 rows.
score(t,n) = 2*ze.c_n - |c_n|^2; argmax_n score == argmin_n d2.
Precision: f32r (hw-rounded fp32) main product plus two f32r residual
products (eps_z.d and z.eps_d) recovers ~fp32-grade scores at 1 PE
cycle/row each; -|c|^2 enters as a K=3 bf16 matmul of a 3-way bf16 split.
argmax: DVE max + max_index per 128-row tile; e gathered with dma_gather.
"""
import sys
if '/opt/trn_rl_repo' not in sys.path:
    sys.path.insert(0, '/opt/trn_rl_repo')

import numpy as np
import ml_dtypes
from contextlib import ExitStack

import concourse.bass as bass
import concourse.bacc as bacc
import concourse.mybir as mybir
from concourse.bass_utils import run_bass_kernel_spmd

B, T, D, N = 32, 2048, 256, 1024
CORES = 8
ROWS = (B // CORES) * T          # 8192 rows per core
NTILES = ROWS // 128             # 64
CHUNK = 8                        # tiles per gather chunk (16 fails on hw dma_gather)
f32 = mybir.dt.float32
f32r = mybir.dt.float32r
bf16 = mybir.dt.bfloat16
u16 = mybir.dt.uint16
i16 = mybir.dt.int16

_CACHE = {}


def build(ntiles=NTILES):
    nchunk = ntiles // CHUNK if ntiles >= CHUNK else 1
    chunk = CHUNK if ntiles >= CHUNK else ntiles
    crow = chunk * 128
    rows = ntiles * 128
    nc = bacc.Bacc()
    ze_d = nc.dram_tensor("ze", [rows, D], f32, kind="ExternalInput")
    dT2_d = nc.dram_tensor("dT2", [D, N], f32, kind="ExternalInput")
    nd3_d = nc.dram_tensor("nd3", [3, N], bf16, kind="ExternalInput")
    ident_d = nc.dram_tensor("ident", [128, 128], f32, kind="ExternalInput")
    dic_d = nc.dram_tensor("dic", [N, D], f32, kind="ExternalInput")
    e_d = nc.dram_tensor("e", [rows, D], f32, kind="ExternalOutput")

    ctx = ExitStack()
    with ctx:
        def sb(name, shape, dt):
            return ctx.enter_context(nc.sbuf_tensor(name, list(shape), dt))

        dT2_sb = sb("dT2_sb", (128, 2, N), f32)
        dr = sb("dr", (128, 2, N), f32r)
        ed_tmp = sb("ed_tmp", (128, 2, N), f32)
        ed = sb("ed", (128, 2, N), f32r)
        nd3_sb = sb("nd3_sb", (3, N), bf16)
        ones3 = sb("ones3", (3, 128), bf16)
        ident_sb = sb("ident_sb", (128, 128), f32)
        ze_nat = [sb(f"ze_nat{p}", (128, D), f32) for p in range(4)]
        zT = [sb(f"zT{p}", (128, 2, 128), f32) for p in range(4)]
        zr = [sb(f"zr{p}", (128, 2, 128), f32r) for p in range(4)]
        ez = [sb(f"ez{p}", (128, 2, 128), f32) for p in range(4)]
        ezr = [sb(f"ezr{p}", (128, 2, 128), f32r) for p in range(4)]
        scores = [sb(f"scores{p}", (128, N), f32) for p in range(2)]
        max8 = [sb(f"max8{p}", (128, 8), f32) for p in range(2)]
        staging = sb("staging", (128, ntiles, 8), u16)
        idxs16 = [sb(f"idxs16{p}", (128, chunk, 8), i16) for p in range(2)]
        gth = [sb(f"gth{p}", (128, chunk, D), f32) for p in range(2)]

        psum_t = [ctx.enter_context(nc.psum_tensor(f"pst{p}", [128, 2, 128], f32))
                  for p in range(4)]
        psum_s = [ctx.enter_context(nc.psum_tensor(f"pss{j}", [128, 512], f32))
                  for j in range(4)]

        sem = {}
        for s in ("prep_dma", "prep_dve", "ze0", "ze1", "ze2", "ze3", "pe_t", "act_t", "gp_ez",
                  "pe_m", "act_s", "act_ez", "dve", "rel", "gth_s", "out0", "out1"):
            sem[s] = ctx.enter_context(nc.semaphore(s))

        with nc.Block() as block:

            @block.sync
            def _(sync):
                sync.dma_start(out=dT2_sb[:], in_=dT2_d.rearrange(
                    "(c p) n -> p c n", p=128)).then_inc(sem["prep_dma"], 16)
                sync.dma_start(out=nd3_sb[:], in_=nd3_d[:]).then_inc(sem["prep_dma"], 16)
                sync.dma_start(out=ident_sb[:], in_=ident_d[:]).then_inc(sem["prep_dma"], 16)
                out_issued = 0
                for i in range(ntiles):
                    if i >= 4:
                        sync.wait_ge(sem["pe_t"], i - 3)
                    sync.dma_start(
                        out=ze_nat[i % 4][:],
                        in_=ze_d[i * 128:(i + 1) * 128, :],
                    ).then_inc(sem[f"ze{i % 4}"], 16)
                    if out_issued < nchunk - 1 and i == chunk * (out_issued + 1) + 8:
                        g = out_issued
                        sync.wait_ge(sem["gth_s"], 16 * (g + 1))
                        sync.dma_start(
                            out=e_d[crow * g:crow * (g + 1), :].rearrange(
                                "(c p) d -> p c d", p=128),
                            in_=gth[g % 2][:],
                        ).then_inc(sem[f"out{g % 2}"], 16)
                        out_issued += 1
                for g in range(out_issued, nchunk):
                    sync.wait_ge(sem["gth_s"], 16 * (g + 1))
                    sync.dma_start(
                        out=e_d[crow * g:crow * (g + 1), :].rearrange(
                            "(c p) d -> p c d", p=128),
                        in_=gth[g % 2][:],
                    ).then_inc(sem[f"out{g % 2}"], 16)
                sync.wait_ge(sem["out0"], 16 * ((nchunk + 1) // 2))
                if nchunk > 1:
                    sync.wait_ge(sem["out1"], 16 * (nchunk // 2))

            @block.vector
            def _(vector):
                # one-time dict prep: f32r rounding + residual
                vector.wait_ge(sem["prep_dma"], 48)
                vector.tensor_copy(dr[:], dT2_sb[:])
                vector.memset(ones3[:], 1.0)
                vector.drain()
                vector.scalar_tensor_tensor(
                    out=ed_tmp[:], in0=dr[:], scalar=-1.0, in1=dT2_sb[:],
                    op0=mybir.AluOpType.mult, op1=mybir.AluOpType.add)
                vector.drain()
                vector.tensor_copy(ed[:], ed_tmp[:]).then_inc(sem["prep_dve"], 1)
                for i in range(ntiles):
                    p = i % 2
                    vector.wait_ge(sem["act_s"], 2 * i + 2)
                    vector.max(max8[p][:], scores[p][:])
                    vector.drain()
                    vector.max_index(staging[:, i, :], max8[p][:],
                                     scores[p][:]).then_inc(sem["dve"], 1)

            @block.tensor
            def _(tensor):
                def emit_T(i):
                    p = i % 4
                    tensor.wait_ge(sem[f"ze{i % 4}"], 16 * (i // 4 + 1))
                    if i >= 4:
                        tensor.wait_ge(sem["act_t"], 2 * i - 6)
                    for c in range(2):
                        mm = tensor.matmul(psum_t[p][:, c, :],
                                           ze_nat[p][:, c * 128:(c + 1) * 128],
                                           ident_sb[:], is_transpose=True,
                                           start=True, stop=True)
                    mm.then_inc(sem["pe_t"], 1)

                tensor.wait_ge(sem["prep_dma"], 48)
                tensor.wait_ge(sem["prep_dve"], 1)
                for j in range(min(4, ntiles)):
                    emit_T(j)
                for i in range(ntiles):
                    p = i % 4
                    tensor.wait_ge(sem["act_ez"], i + 1)
                    for nt in range(2):
                        ps = psum_s[2 * (i % 2) + nt]
                        if i >= 2:
                            tensor.wait_ge(sem["act_s"], 2 * (i - 2) + nt + 1)
                        ns = bass.ts(nt, 512)
                        tensor.matmul(ps[:], zr[p][:, 0, :], dr[:, 0, ns],
                                      start=True, stop=False)
                        tensor.matmul(ps[:], zr[p][:, 1, :], dr[:, 1, ns],
                                      start=False, stop=False)
                        tensor.matmul(ps[:], ezr[p][:, 0, :], dr[:, 0, ns],
                                      start=False, stop=False)
                        tensor.matmul(ps[:], ezr[p][:, 1, :], dr[:, 1, ns],
                                      start=False, stop=False)
                        tensor.matmul(ps[:], zr[p][:, 0, :], ed[:, 0, ns],
                                      start=False, stop=False)
                        tensor.matmul(ps[:], zr[p][:, 1, :], ed[:, 1, ns],
                                      start=False, stop=False)
                        tensor.matmul(ps[:], ones3[:], nd3_sb[:, ns],
                                      start=False, stop=True).then_inc(sem["pe_m"], 1)
                    if i + 4 < ntiles:
                        emit_T(i + 4)

            @block.scalar
            def _(scalar):
                def copy_tz(i):
                    p = i % 4
                    scalar.wait_ge(sem["pe_t"], i + 1)
                    if i >= 4:
                        scalar.wait_ge(sem["act_ez"], i - 3)
                        scalar.wait_ge(sem["pe_m"], 2 * i - 6)
                    scalar.copy(zT[p][:], psum_t[p][:]).then_inc(sem["act_t"], 1)
                    scalar.copy(zr[p][:], psum_t[p][:]).then_inc(sem["act_t"], 1)

                def round_ez(i):
                    p = i % 4
                    scalar.wait_ge(sem["gp_ez"], i + 1)
                    scalar.copy(ezr[p][:], ez[p][:]).then_inc(sem["act_ez"], 1)

                for j in range(min(3, ntiles)):
                    copy_tz(j)
                for j in range(min(2, ntiles)):
                    round_ez(j)
                for i in range(ntiles):
                    p = i % 2
                    if i + 3 < ntiles:
                        copy_tz(i + 3)
                    if i + 2 < ntiles:
                        round_ez(i + 2)
                    if i >= 2:
                        scalar.wait_ge(sem["dve"], i - 1)
                    for nt in range(2):
                        scalar.wait_ge(sem["pe_m"], 2 * i + nt + 1)
                        scalar.copy(scores[p][:, bass.ts(nt, 512)],
                                    psum_s[2 * p + nt][:]).then_inc(sem["act_s"], 1)

            @block.gpsimd
            def _(gpsimd):
                def emit_chunk(g):
                    if True:
                        q = g % 2
                        gpsimd.wait_ge(sem["dve"], chunk * (g + 1))
                        if g >= 2:
                            gpsimd.wait_ge(sem["gth_s"], 16 * (g - 1))
                        with nc.allow_non_contiguous_dma(reason="16x2B idx relayout"):
                            for k in range(8):
                                gpsimd.dma_start(
                                    out=idxs16[q][0:16, :, k:k + 1],
                                    in_=staging[k * 16:(k + 1) * 16,
                                                chunk * g:chunk * (g + 1),
                                                0:1].bitcast(i16),
                                ).then_inc(sem["rel"], 16)
                        gpsimd.wait_ge(sem["rel"], 240 * g + 128)
                        for r in range(1, 8):
                            gpsimd.dma_start(
                                out=idxs16[q][16 * r:16 * (r + 1), :, :],
                                in_=idxs16[q][0:16, :, :],
                            ).then_inc(sem["rel"], 16)
                        gpsimd.wait_ge(sem["rel"], 240 * (g + 1))
                        if g >= 2:
                            gpsimd.wait_ge(sem[f"out{g % 2}"], 16 * (g // 2))
                        if g >= 1:
                            gpsimd.wait_ge(sem["gth_s"], 16 * g)
                        gpsimd.dma_gather(
                            out_ap=gth[q][:],
                            in_ap=dic_d[:],
                            idxs_ap=idxs16[q][:],
                            num_idxs=crow,
                            num_idxs_reg=crow,
                            elem_size=D,
                            elem_step=D,
                        ).then_inc(sem["gth_s"], 16)

                done_chunks = 0
                for i in range(ntiles):
                    gpsimd.wait_ge(sem["act_t"], 2 * i + 2)
                    if i >= 4:
                        gpsimd.wait_ge(sem["act_ez"], i - 3)
                    gpsimd.tensor_sub(ez[i % 4][:], zT[i % 4][:], zr[i % 4][:]).then_inc(sem["gp_ez"], 1)
                    if i >= chunk + 1 and (i - 1) % chunk == 0 and done_chunks < (i - 1) // chunk:
                        emit_chunk(done_chunks)
                        done_chunks += 1
                for g in range(done_chunks, nchunk):
                    emit_chunk(g)

    nc.finalize()
    return nc


def _prep_host(dictionary):
    dic = np.ascontiguousarray(dictionary.astype(np.float32))
    dT2 = np.ascontiguousarray(2.0 * dic.T).astype(np.float32)
    nd = -(dic.astype(np.float64) ** 2).sum(-1)
    h1 = nd.astype(ml_dtypes.bfloat16)
    r1 = nd - h1.astype(np.float64)
    h2 = r1.astype(ml_dtypes.bfloat16)
    r2 = r1 - h2.astype(np.float64)
    h3 = r2.astype(ml_dtypes.bfloat16)
    nd3 = np.stack([h1, h2, h3]).astype(ml_dtypes.bfloat16)
    ident = np.eye(128, dtype=np.float32)
    return dic, dT2, nd3, ident


def kernel(ze, dictionary):
    if "nc" not in _CACHE:
        _CACHE["nc"] = build()
    nc = _CACHE["nc"]
    dic, dT2, nd3, ident = _prep_host(dictionary)
    ze = np.ascontiguousarray(np.asarray(ze, dtype=np.float32))
    zec = ze.reshape(CORES, ROWS, D)
    in_maps = [{"ze": np.ascontiguousarray(zec[c]), "dT2": dT2, "nd3": nd3,
                "ident": ident, "dic": dic} for c in range(CORES)]
    res = run_bass_kernel_spmd(nc, in_maps, list(range(CORES)))
    e = np.stack([res.results[c]["e"] for c in range(CORES)])
    return e.reshape(B, T, D)



# revision 22
# speedup vs baseline: 2.1198x; 2.1198x over previous
"""VQ codebook-lookup kernel for Trainium2 (8 NeuronCores, data-parallel over batch).

e[b,t,:] = dictionary[argmin_n ||ze[b,t,:] - dictionary[n,:]||^2]

Per core: 4 batches x 2048 rows = 8192, tiled 64 x 128 rows.
score(t,n) = 2*ze.c_n - |c_n|^2; argmax_n score == argmin_n d2.

Host splits ze and 2*dict^T into bf16 hi/lo pairs (pre-transposed), giving
~18-bit operands; scores = zh.dh + zh.dl + zl.dh accumulated in fp32 PSUM on
top of an Act-preloaded -|c|^2 bias row (matmuls run start=False). No PE
transposes, no norm matmul: 12 bf16 MMs per 128-row tile is the whole PE load.
Act copies scores PSUM->SBUF; DVE max + max_index give the argmax. Index
relayout to the 16-partition gather format runs on Pool SWDGE for the early
batches (keeps SP's HWDGE path free for loads) and on SP for the tail; Pool
dma_gather fetches the codebook rows.
"""
import sys
if '/opt/trn_rl_repo' not in sys.path:
    sys.path.insert(0, '/opt/trn_rl_repo')

import numpy as np
import ml_dtypes
from contextlib import ExitStack

import concourse.bass as bass
import concourse.bacc as bacc
import concourse.mybir as mybir
from concourse.bass_utils import run_bass_kernel_spmd

B, T, D, N = 32, 2048, 256, 1024
CORES = 8
ROWS = (B // CORES) * T          # 8192 rows per core
NTILES = ROWS // 128             # 64
CHUNK = 8                        # tiles per gather chunk
REPLICATE = True
NZBUF = 8                        # zhl pair-load prefetch depth (16 tiles)

f32 = mybir.dt.float32
bf16 = mybir.dt.bfloat16
u16 = mybir.dt.uint16
i16 = mybir.dt.int16

_CACHE = {}

TERMS = [(0, 0), (1, 0), (0, 1)]  # (z plane, d plane): hh, lh, hl — dl last
                                  # so the split dict-lo load can arrive late


def build(ntiles=NTILES):
    nchunk = ntiles // CHUNK
    ndma_b = 8 + (7 if REPLICATE else 0)
    # per-chunk relayout into offset-0 ping-pong buffers (the only layout the
    # hw gather path tolerates). Chunks 0..nchunk-2 relayout on Pool SWDGE;
    # the final chunk's relayout runs on SP HWDGE at the tail.
    pool_chunks = list(range(nchunk))
    sp_chunks = []
    rel_sem = {}
    rel_after = {}
    rel_base = {}
    for group, name in ((pool_chunks, "relp"), (sp_chunks, "rels")):
        for j, g in enumerate(group):
            rel_sem[g] = name
            rel_base[g] = 16 * ndma_b * j
            rel_after[g] = 16 * ndma_b * (j + 1)
    store_pos = {}
    inline_stores = set()
    for g in range(nchunk):
        pos = CHUNK * g + 34
        if pos < ntiles:
            store_pos.setdefault(pos, []).append(g)
            inline_stores.add(g)
    # SP tail: leftover stores ascending; the final chunk's relayout is
    # emitted one store earlier than strictly needed (it only gates on dve,
    # which resolves before the preceding store's gather does)
    tail_stores = [g for g in range(nchunk) if g not in inline_stores]
    tail = []
    emitted = set(pool_chunks)
    for j, g in enumerate(tail_stores):
        nxt = tail_stores[j + 1] if j + 1 < len(tail_stores) else None
        if nxt is not None and nxt in sp_chunks and nxt not in emitted:
            tail.append(("relayout", nxt))
            emitted.add(nxt)
        tail.append(("store", g))
    for g in sp_chunks:
        if g not in emitted:
            tail.append(("relayout", g))

    nc = bacc.Bacc()
    zhl_d = nc.dram_tensor("zhl", [2, 2, 128, ntiles * 128], bf16, kind="ExternalInput")
    nd3_d = nc.dram_tensor("nd3", [3, N], bf16, kind="ExternalInput")
    dhl_d = nc.dram_tensor("dhl", [2, 2, 128, N], bf16, kind="ExternalInput")
    nb_d = nc.dram_tensor("nb", [128, N], f32, kind="ExternalInput")
    dic_d = nc.dram_tensor("dic", [N, D], f32, kind="ExternalInput")
    e_d = nc.dram_tensor("e", [ntiles * 128, D], f32, kind="ExternalOutput")

    npair = (ntiles + 1) // 2

    ctx = ExitStack()
    with ctx:
        def sb(name, shape, dt):
            return ctx.enter_context(nc.sbuf_tensor(name, list(shape), dt))

        zhl = [sb(f"zhl{p}", (128, 2, 2, 256), bf16) for p in range(NZBUF)]
        dhl = sb("dhl_sb", (128, 2, 2, N), bf16)
        nb = sb("nb_sb", (128, N), f32)
        nd3_sb = sb("nd3_sb", (3, N), bf16)
        ones3 = sb("ones3", (3, 128), bf16)
        scores = [sb(f"scores{p}", (128, N), f32) for p in range(4)]
        m8 = [sb(f"m8_{p}", (128, 8), f32) for p in range(2)]
        staging = sb("staging", (128, ntiles, 8), u16)
        idxs2 = [sb(f"idxs{q}", (128, CHUNK, 8), i16) for q in range(2)]
        gth = [sb(f"gth{q}", (128, CHUNK, D), f32) for q in range(2)]
        ps = [ctx.enter_context(nc.psum_tensor(f"ps{j}", [128, N], f32))
              for j in range(4)]

        sem = {}
        for s in ("prepd", "prepdl", "prepn", "prep3", "m1", "z", "actb",
                  "pem", "acts", "dve", "relp", "rels", "gth", "outs"):
            sem[s] = ctx.enter_context(nc.semaphore(s))

        def relayout(eng, g):
            s, e = CHUNK * g, CHUNK * (g + 1)
            q = g % 2
            rs = sem[rel_sem[g]]
            eng.wait_ge(sem["dve"], e)
            if g >= 2:
                eng.wait_ge(sem["gth"], 16 * (g - 1))
            with nc.allow_non_contiguous_dma(reason="16x2B idx relayout"):
                for kk in range(8):
                    eng.dma_start(
                        out=idxs2[q][0:16, :, kk:kk + 1],
                        in_=staging[16 * kk:16 * (kk + 1), s:e, 0:1].bitcast(i16),
                    ).then_inc(rs, 16)
            if REPLICATE:
                eng.wait_ge(rs, rel_base[g] + 16 * 8)
                for r in range(1, 8):
                    eng.dma_start(
                        out=idxs2[q][16 * r:16 * (r + 1), :, :],
                        in_=idxs2[q][0:16, :, :],
                    ).then_inc(rs, 16)

        def store(sync, g):
            sync.wait_ge(sem["gth"], 16 * (g + 1))
            sync.dma_start(
                out=e_d[CHUNK * 128 * g:CHUNK * 128 * (g + 1), :].rearrange(
                    "(c p) d -> p c d", p=128),
                in_=gth[g % 2][:],
            ).then_inc(sem["outs"], 16)

        with nc.Block() as block:

            @block.sync
            def _(sync):
                sync.dma_start(out=nb[:], in_=nb_d[:]).then_inc(sem["prepn"], 16)
                sync.dma_start(out=nd3_sb[:], in_=nd3_d[:]).then_inc(sem["prep3"], 16)
                sync.dma_start(
                    out=zhl[0][:],
                    in_=zhl_d[:, :, :, 0:256].rearrange("s c p t -> p s c t"),
                ).then_inc(sem["z"], 16)
                sync.dma_start(out=dhl[:, 0, :, :],
                               in_=dhl_d[0].rearrange("c p n -> p c n")
                               ).then_inc(sem["prepd"], 16)
                sync.dma_start(out=dhl[:, 1, :, :],
                               in_=dhl_d[1].rearrange("c p n -> p c n")
                               ).then_inc(sem["prepdl"], 16)
                for j in range(1, npair):
                    i = 2 * j
                    if i >= 2 * NZBUF:
                        sync.wait_ge(sem["pem"], 2 * (i - 2 * NZBUF) + 4)
                    sync.dma_start(
                        out=zhl[j % NZBUF][:],
                        in_=zhl_d[:, :, :, i * 128:(i + 2) * 128].rearrange(
                            "s c p t -> p s c t"),
                    ).then_inc(sem["z"], 16)
                    for g in store_pos.get(i, []) + store_pos.get(i + 1, []):
                        store(sync, g)
                for kind, v in tail:
                    if kind == "relayout":
                        relayout(sync, v)
                    else:
                        store(sync, v)
                sync.wait_ge(sem["outs"], 16 * nchunk)

            @block.scalar
            def _(scalar):
                scalar.wait_ge(sem["prepn"], 16)
                for i in range(ntiles):
                    p = i % 4
                    if i >= 4:
                        scalar.wait_ge(sem["dve"], i - 3)
                    scalar.wait_ge(sem["pem"], 2 * i + 2)
                    scalar.copy(scores[p][:], ps[p][:]).then_inc(sem["acts"], 1)
                    if i + 4 < ntiles:
                        scalar.drain()
                        scalar.copy(ps[p][:], nb[:]).then_inc(sem["actb"], 1)

            @block.tensor
            def _(tensor):
                tensor.wait_ge(sem["prepd"], 16)
                tensor.wait_ge(sem["prep3"], 16)
                tensor.wait_ge(sem["m1"], 1)
                for i in range(ntiles):
                    p = i % 4
                    q = (i // 2) % NZBUF
                    tsl = slice(128 * (i % 2), 128 * (i % 2) + 128)
                    tensor.wait_ge(sem["z"], 16 * (i // 2 + 1))
                    if i >= 4:
                        tensor.wait_ge(sem["actb"], i - 3)
                    for h in range(2):
                        ns = bass.ts(h, 512)
                        mm = None
                        if i < 4:
                            tensor.matmul(ps[p][:, ns], ones3[:], nd3_sb[:, ns],
                                          start=True, stop=False,
                                          skip_group_check=True)
                        for t, (sz, sd) in enumerate(TERMS):
                            if i == 0 and h == 0 and sd == 1:
                                tensor.wait_ge(sem["prepdl"], 16)
                            for c in range(2):
                                mm = tensor.matmul(ps[p][:, ns],
                                                   zhl[q][:, sz, c, tsl],
                                                   dhl[:, sd, c, ns],
                                                   start=False,
                                                   stop=(t == 2 and c == 1),
                                                   skip_group_check=True)
                        mm.then_inc(sem["pem"], 1)

            @block.vector
            def _(vector):
                vector.memset(ones3[:], 1.0)
                vector.drain()
                vector.nop().then_inc(sem["m1"], 1)
                for i in range(ntiles):
                    p = i % 4
                    vector.wait_ge(sem["acts"], i + 1)
                    vector.max(m8[i % 2][:], scores[p][:])
                    vector.drain()
                    vector.max_index(staging[:, i, :], m8[i % 2][:],
                                     scores[p][:]).then_inc(sem["dve"], 1)

            @block.gpsimd
            def _(gpsimd):
                for g in range(nchunk):
                    if g in pool_chunks:
                        relayout(gpsimd, g)
                    gpsimd.wait_ge(sem[rel_sem[g]], rel_after[g])
                    if g >= 2:
                        gpsimd.wait_ge(sem["outs"], 16 * (g - 1))
                    if g >= 1:
                        gpsimd.wait_ge(sem["gth"], 16 * g)
                    gpsimd.dma_gather(
                        out_ap=gth[g % 2][:],
                        in_ap=dic_d[:],
                        idxs_ap=idxs2[g % 2][:],
                        num_idxs=CHUNK * 128,
                        num_idxs_reg=CHUNK * 128,
                        elem_size=D,
                        elem_step=D,
                    ).then_inc(sem["gth"], 16)

    nc.finalize()
    return nc


def _prep_dict(dictionary):
    dic = np.ascontiguousarray(dictionary.astype(np.float32))
    dT2 = np.ascontiguousarray(2.0 * dic.T).astype(np.float32)   # [D, N]
    dh = dT2.astype(ml_dtypes.bfloat16)
    dl = (dT2 - dh.astype(np.float32)).astype(ml_dtypes.bfloat16)
    dhl = np.ascontiguousarray(
        np.stack([dh.reshape(2, 128, N), dl.reshape(2, 128, N)]))
    nd = -(dic.astype(np.float64) ** 2).sum(-1)
    nb = np.ascontiguousarray(
        np.broadcast_to(nd.astype(np.float32), (128, N)))
    h1 = nd.astype(ml_dtypes.bfloat16)
    r1 = nd - h1.astype(np.float64)
    h2 = r1.astype(ml_dtypes.bfloat16)
    r2 = r1 - h2.astype(np.float64)
    h3 = r2.astype(ml_dtypes.bfloat16)
    nd3 = np.ascontiguousarray(np.stack([h1, h2, h3]).astype(ml_dtypes.bfloat16))
    return dic, dhl, nb, nd3


def _prep_ze(ze_core):
    zh = ze_core.astype(ml_dtypes.bfloat16)
    zl = (ze_core - zh.astype(np.float32)).astype(ml_dtypes.bfloat16)
    rows = ze_core.shape[0]
    return np.ascontiguousarray(
        np.stack([np.ascontiguousarray(zh.T).reshape(2, 128, rows),
                  np.ascontiguousarray(zl.T).reshape(2, 128, rows)]))


def kernel(ze, dictionary):
    if "nc" not in _CACHE:
        _CACHE["nc"] = build()
    nc = _CACHE["nc"]
    dic, dhl, nb, nd3 = _prep_dict(np.asarray(dictionary))
    ze = np.ascontiguousarray(np.asarray(ze, dtype=np.float32))
    zec = ze.reshape(CORES, ROWS, D)
    in_maps = [{"zhl": _prep_ze(zec[c]), "dhl": dhl, "nb": nb, "nd3": nd3,
                "dic": dic} for c in range(CORES)]
    res = run_bass_kernel_spmd(nc, in_maps, list(range(CORES)))
    e = np.stack([res.results[c]["e"] for c in range(CORES)])
    return e.reshape(B, T, D)


# revision 24
# speedup vs baseline: 2.1818x; 1.0293x over previous
"""VQ codebook-lookup kernel for Trainium2 (8 NeuronCores, data-parallel over batch).

e[b,t,:] = dictionary[argmin_n ||ze[b,t,:] - dictionary[n,:]||^2]

Per core: 4 batches x 2048 rows = 8192, tiled 64 x 128 rows.
score(t,n) = 2*ze.c_n - |c_n|^2; argmax_n score == argmin_n d2.

Host splits ze and 2*dict^T into bf16 hi/lo pairs (pre-transposed), giving
~18-bit operands; scores = zh.dh + zh.dl + zl.dh accumulated in fp32 PSUM on
top of an Act-preloaded -|c|^2 bias row (matmuls run start=False). No PE
transposes, no norm matmul: 12 bf16 MMs per 128-row tile is the whole PE load.
Act copies scores PSUM->SBUF; DVE max + max_index give the argmax. Index
relayout to the 16-partition gather format runs on Pool SWDGE for the early
batches (keeps SP's HWDGE path free for loads) and on SP for the tail; Pool
dma_gather fetches the codebook rows.
"""
import sys
if '/opt/trn_rl_repo' not in sys.path:
    sys.path.insert(0, '/opt/trn_rl_repo')

import numpy as np
import ml_dtypes
from contextlib import ExitStack

import concourse.bass as bass
import concourse.bacc as bacc
import concourse.mybir as mybir
from concourse.bass_utils import run_bass_kernel_spmd

B, T, D, N = 32, 2048, 256, 1024
CORES = 8
ROWS = (B // CORES) * T          # 8192 rows per core
NTILES = ROWS // 128             # 64
CHUNK = 8                        # tiles per gather chunk
REPLICATE = True
NZBUF = 8                        # zhl pair-load prefetch depth (16 tiles)

f32 = mybir.dt.float32
bf16 = mybir.dt.bfloat16
u16 = mybir.dt.uint16
i16 = mybir.dt.int16

_CACHE = {}

TERMS = [(0, 0), (1, 0), (0, 1)]  # (z plane, d plane): hh, lh, hl — dl last
                                  # so the split dict-lo load can arrive late


def build(ntiles=NTILES):
    nchunk = ntiles // CHUNK
    ndma_b = 8 + (7 if REPLICATE else 0)
    # per-chunk relayout into offset-0 ping-pong buffers (the only layout the
    # hw gather path tolerates). Chunks 0..nchunk-2 relayout on Pool SWDGE;
    # the final chunk's relayout runs on SP HWDGE at the tail.
    pool_chunks = list(range(nchunk - 1))
    sp_chunks = [nchunk - 1] if nchunk else []
    rel_sem = {}
    rel_after = {}
    rel_base = {}
    for group, name in ((pool_chunks, "relp"), (sp_chunks, "rels")):
        for j, g in enumerate(group):
            rel_sem[g] = name
            rel_base[g] = 16 * ndma_b * j
            rel_after[g] = 16 * ndma_b * (j + 1)
    store_pos = {}
    inline_stores = set()
    for g in range(nchunk):
        pos = CHUNK * g + 34
        if pos < ntiles:
            store_pos.setdefault(pos, []).append(g)
            inline_stores.add(g)
    # SP tail: leftover stores ascending; the final chunk's relayout is
    # emitted one store earlier than strictly needed (it only gates on dve,
    # which resolves before the preceding store's gather does)
    tail_stores = [g for g in range(nchunk) if g not in inline_stores]
    tail = []
    emitted = set(pool_chunks)
    for j, g in enumerate(tail_stores):
        nxt = tail_stores[j + 1] if j + 1 < len(tail_stores) else None
        if nxt is not None and nxt in sp_chunks and nxt not in emitted:
            tail.append(("relayout", nxt))
            emitted.add(nxt)
        tail.append(("store", g))
    for g in sp_chunks:
        if g not in emitted:
            tail.append(("relayout", g))

    nc = bacc.Bacc()
    zhl_d = nc.dram_tensor("zhl", [2, 2, 128, ntiles * 128], bf16, kind="ExternalInput")
    nd3_d = nc.dram_tensor("nd3", [3, N], bf16, kind="ExternalInput")
    dhl_d = nc.dram_tensor("dhl", [2, 2, 128, N], bf16, kind="ExternalInput")
    nb_d = nc.dram_tensor("nb", [128, N], f32, kind="ExternalInput")
    dic_d = nc.dram_tensor("dic", [N, D], f32, kind="ExternalInput")
    e_d = nc.dram_tensor("e", [ntiles * 128, D], f32, kind="ExternalOutput")

    npair = (ntiles + 1) // 2

    ctx = ExitStack()
    with ctx:
        def sb(name, shape, dt):
            return ctx.enter_context(nc.sbuf_tensor(name, list(shape), dt))

        zhl = [sb(f"zhl{p}", (128, 2, 2, 256), bf16) for p in range(NZBUF)]
        dhl = sb("dhl_sb", (128, 2, 2, N), bf16)
        nb = sb("nb_sb", (128, N), f32)
        nd3_sb = sb("nd3_sb", (3, N), bf16)
        ones3 = sb("ones3", (3, 128), bf16)
        scores = [sb(f"scores{p}", (128, N), f32) for p in range(4)]
        m8 = [sb(f"m8_{p}", (128, 8), f32) for p in range(2)]
        staging = sb("staging", (128, ntiles, 8), u16)
        idxs2 = [sb(f"idxs{q}", (128, CHUNK, 8), i16) for q in range(2)]
        gth = [sb(f"gth{q}", (128, CHUNK, D), f32) for q in range(2)]
        ps = [ctx.enter_context(nc.psum_tensor(f"ps{j}", [128, N], f32))
              for j in range(4)]

        sem = {}
        for s in ("prepd", "prepdl", "prepn", "prep3", "m1", "z", "actb",
                  "pem", "acts", "dve", "relp", "rels", "gth", "outs"):
            sem[s] = ctx.enter_context(nc.semaphore(s))

        def relayout(eng, g):
            s, e = CHUNK * g, CHUNK * (g + 1)
            q = g % 2
            rs = sem[rel_sem[g]]
            eng.wait_ge(sem["dve"], e)
            if g >= 2:
                eng.wait_ge(sem["gth"], 16 * (g - 1))
            with nc.allow_non_contiguous_dma(reason="16x2B idx relayout"):
                for kk in range(8):
                    eng.dma_start(
                        out=idxs2[q][0:16, :, kk:kk + 1],
                        in_=staging[16 * kk:16 * (kk + 1), s:e, 0:1].bitcast(i16),
                    ).then_inc(rs, 16)
            if REPLICATE:
                eng.wait_ge(rs, rel_base[g] + 16 * 8)
                for r in range(1, 8):
                    eng.dma_start(
                        out=idxs2[q][16 * r:16 * (r + 1), :, :],
                        in_=idxs2[q][0:16, :, :],
                    ).then_inc(rs, 16)

        def store(sync, g):
            sync.wait_ge(sem["gth"], 16 * (g + 1))
            sync.dma_start(
                out=e_d[CHUNK * 128 * g:CHUNK * 128 * (g + 1), :].rearrange(
                    "(c p) d -> p c d", p=128),
                in_=gth[g % 2][:],
            ).then_inc(sem["outs"], 16)

        with nc.Block() as block:

            @block.sync
            def _(sync):
                sync.dma_start(out=nb[:], in_=nb_d[:]).then_inc(sem["prepn"], 16)
                sync.dma_start(out=nd3_sb[:], in_=nd3_d[:]).then_inc(sem["prep3"], 16)
                sync.dma_start(
                    out=zhl[0][:],
                    in_=zhl_d[:, :, :, 0:256].rearrange("s c p t -> p s c t"),
                ).then_inc(sem["z"], 16)
                sync.dma_start(out=dhl[:, 0, :, :],
                               in_=dhl_d[0].rearrange("c p n -> p c n")
                               ).then_inc(sem["prepd"], 16)
                sync.dma_start(out=dhl[:, 1, :, :],
                               in_=dhl_d[1].rearrange("c p n -> p c n")
                               ).then_inc(sem["prepdl"], 16)
                for j in range(1, npair):
                    i = 2 * j
                    if i >= 2 * NZBUF:
                        sync.wait_ge(sem["pem"], 2 * (i - 2 * NZBUF) + 4)
                    sync.dma_start(
                        out=zhl[j % NZBUF][:],
                        in_=zhl_d[:, :, :, i * 128:(i + 2) * 128].rearrange(
                            "s c p t -> p s c t"),
                    ).then_inc(sem["z"], 16)
                    for g in store_pos.get(i, []) + store_pos.get(i + 1, []):
                        store(sync, g)
                for kind, v in tail:
                    if kind == "relayout":
                        relayout(sync, v)
                    else:
                        store(sync, v)
                sync.wait_ge(sem["outs"], 16 * nchunk)

            @block.scalar
            def _(scalar):
                scalar.wait_ge(sem["prepn"], 16)
                for i in range(ntiles):
                    p = i % 4
                    if i >= 4:
                        scalar.wait_ge(sem["dve"], i - 3)
                    scalar.wait_ge(sem["pem"], 2 * i + 2)
                    scalar.copy(scores[p][:], ps[p][:]).then_inc(sem["acts"], 1)
                    if i + 4 < ntiles:
                        scalar.drain()
                        scalar.copy(ps[p][:], nb[:]).then_inc(sem["actb"], 1)

            @block.tensor
            def _(tensor):
                tensor.wait_ge(sem["prepd"], 16)
                tensor.wait_ge(sem["prep3"], 16)
                tensor.wait_ge(sem["m1"], 1)
                for i in range(ntiles):
                    p = i % 4
                    q = (i // 2) % NZBUF
                    tsl = slice(128 * (i % 2), 128 * (i % 2) + 128)
                    tensor.wait_ge(sem["z"], 16 * (i // 2 + 1))
                    if i >= 4:
                        tensor.wait_ge(sem["actb"], i - 3)
                    for h in range(2):
                        ns = bass.ts(h, 512)
                        mm = None
                        if i < 4:
                            tensor.matmul(ps[p][:, ns], ones3[:], nd3_sb[:, ns],
                                          start=True, stop=False,
                                          skip_group_check=True)
                        for t, (sz, sd) in enumerate(TERMS):
                            if i == 0 and h == 0 and sd == 1:
                                tensor.wait_ge(sem["prepdl"], 16)
                            for c in range(2):
                                mm = tensor.matmul(ps[p][:, ns],
                                                   zhl[q][:, sz, c, tsl],
                                                   dhl[:, sd, c, ns],
                                                   start=False,
                                                   stop=(t == 2 and c == 1),
                                                   skip_group_check=True)
                        mm.then_inc(sem["pem"], 1)

            @block.vector
            def _(vector):
                vector.memset(ones3[:], 1.0)
                vector.drain()
                vector.nop().then_inc(sem["m1"], 1)
                for i in range(ntiles):
                    p = i % 4
                    vector.wait_ge(sem["acts"], i + 1)
                    vector.max(m8[i % 2][:], scores[p][:])
                    vector.drain()
                    vector.max_index(staging[:, i, :], m8[i % 2][:],
                                     scores[p][:]).then_inc(sem["dve"], 1)

            @block.gpsimd
            def _(gpsimd):
                reg = gpsimd.to_reg(CHUNK * 128)
                for g in range(nchunk):
                    if g in pool_chunks:
                        relayout(gpsimd, g)
                    gpsimd.wait_ge(sem[rel_sem[g]], rel_after[g])
                    if g >= 2:
                        gpsimd.wait_ge(sem["outs"], 16 * (g - 1))
                    if g >= 1:
                        gpsimd.wait_ge(sem["gth"], 16 * g)
                    gpsimd.dma_gather(
                        out_ap=gth[g % 2][:],
                        in_ap=dic_d[:],
                        idxs_ap=idxs2[g % 2][:],
                        num_idxs=CHUNK * 128,
                        num_idxs_reg=reg,
                        elem_size=D,
                        elem_step=D,
                    ).then_inc(sem["gth"], 16)

    nc.finalize()
    return nc


def _prep_dict(dictionary):
    dic = np.ascontiguousarray(dictionary.astype(np.float32))
    dT2 = np.ascontiguousarray(2.0 * dic.T).astype(np.float32)   # [D, N]
    dh = dT2.astype(ml_dtypes.bfloat16)
    dl = (dT2 - dh.astype(np.float32)).astype(ml_dtypes.bfloat16)
    dhl = np.ascontiguousarray(
        np.stack([dh.reshape(2, 128, N), dl.reshape(2, 128, N)]))
    nd = -(dic.astype(np.float64) ** 2).sum(-1)
    nb = np.ascontiguousarray(
        np.broadcast_to(nd.astype(np.float32), (128, N)))
    h1 = nd.astype(ml_dtypes.bfloat16)
    r1 = nd - h1.astype(np.float64)
    h2 = r1.astype(ml_dtypes.bfloat16)
    r2 = r1 - h2.astype(np.float64)
    h3 = r2.astype(ml_dtypes.bfloat16)
    nd3 = np.ascontiguousarray(np.stack([h1, h2, h3]).astype(ml_dtypes.bfloat16))
    return dic, dhl, nb, nd3


def _prep_ze(ze_core):
    zh = ze_core.astype(ml_dtypes.bfloat16)
    zl = (ze_core - zh.astype(np.float32)).astype(ml_dtypes.bfloat16)
    rows = ze_core.shape[0]
    return np.ascontiguousarray(
        np.stack([np.ascontiguousarray(zh.T).reshape(2, 128, rows),
                  np.ascontiguousarray(zl.T).reshape(2, 128, rows)]))


def kernel(ze, dictionary):
    if "nc" not in _CACHE:
        _CACHE["nc"] = build()
    nc = _CACHE["nc"]
    dic, dhl, nb, nd3 = _prep_dict(np.asarray(dictionary))
    ze = np.ascontiguousarray(np.asarray(ze, dtype=np.float32))
    zec = ze.reshape(CORES, ROWS, D)
    in_maps = [{"zhl": _prep_ze(zec[c]), "dhl": dhl, "nb": nb, "nd3": nd3,
                "dic": dic} for c in range(CORES)]
    res = run_bass_kernel_spmd(nc, in_maps, list(range(CORES)))
    e = np.stack([res.results[c]["e"] for c in range(CORES)])
    return e.reshape(B, T, D)


# revision 27
# speedup vs baseline: 2.2846x; 1.0471x over previous
"""VQ codebook-lookup kernel for Trainium2 (8 NeuronCores, data-parallel over batch).

e[b,t,:] = dictionary[argmin_n ||ze[b,t,:] - dictionary[n,:]||^2]

Per core: 4 batches x 2048 rows = 8192, tiled 64 x 128 rows.
score(t,n) = 2*ze.c_n - |c_n|^2; argmax_n score == argmin_n d2.

Host splits ze and 2*dict^T into bf16 hi/lo pairs (pre-transposed), giving
~18-bit operands; scores = zh.dh + zh.dl + zl.dh accumulated in fp32 PSUM on
top of an Act-preloaded -|c|^2 bias row (matmuls run start=False). No PE
transposes, no norm matmul: 12 bf16 MMs per 128-row tile is the whole PE load.
Act copies scores PSUM->SBUF; DVE max + max_index give the argmax. Index
relayout to the 16-partition gather format runs on Pool SWDGE for the early
batches (keeps SP's HWDGE path free for loads) and on SP for the tail; Pool
dma_gather fetches the codebook rows.
"""
import sys
if '/opt/trn_rl_repo' not in sys.path:
    sys.path.insert(0, '/opt/trn_rl_repo')

import numpy as np
import ml_dtypes
from contextlib import ExitStack

import concourse.bass as bass
import concourse.bacc as bacc
import concourse.mybir as mybir
from concourse.bass_utils import run_bass_kernel_spmd

B, T, D, N = 32, 2048, 256, 1024
CORES = 8
ROWS = (B // CORES) * T          # 8192 rows per core
NTILES = ROWS // 128             # 64
CHUNK = 8                        # tiles per gather chunk
REPLICATE = True
NZBUF = 8                        # zhl pair-load prefetch depth (16 tiles)

f32 = mybir.dt.float32
bf16 = mybir.dt.bfloat16
u16 = mybir.dt.uint16
i16 = mybir.dt.int16

_CACHE = {}

TERMS = [(0, 0), (1, 0), (0, 1)]  # (z plane, d plane): hh, lh, hl — dl last
                                  # so the split dict-lo load can arrive late


def build(ntiles=NTILES):
    nchunk = ntiles // CHUNK
    ndma_b = 8 + (7 if REPLICATE else 0)
    # per-chunk relayout into offset-0 ping-pong buffers (the only layout the
    # hw gather path tolerates). Chunks 0..nchunk-2 relayout on Pool SWDGE;
    # the final chunk's relayout runs on SP HWDGE at the tail.
    pool_chunks = list(range(nchunk - 1))
    sp_chunks = [nchunk - 1] if nchunk else []
    rel_sem = {}
    rel_after = {}
    rel_base = {}
    for group, name in ((pool_chunks, "relp"), (sp_chunks, "rels")):
        for j, g in enumerate(group):
            rel_sem[g] = name
            rel_base[g] = 16 * ndma_b * j
            rel_after[g] = 16 * ndma_b * (j + 1)
    store_pos = {}
    inline_stores = set()
    for g in range(nchunk):
        pos = CHUNK * g + 34
        if pos < ntiles:
            store_pos.setdefault(pos, []).append(g)
            inline_stores.add(g)
    # SP tail: leftover stores ascending; the final chunk's relayout is
    # emitted one store earlier than strictly needed (it only gates on dve,
    # which resolves before the preceding store's gather does)
    tail_stores = [g for g in range(nchunk) if g not in inline_stores]
    act_stores = tail_stores[-2:] if len(tail_stores) >= 2 else []
    tail = []
    emitted = set(pool_chunks)
    for g in tail_stores:
        if g in act_stores:
            continue
        tail.append(("store", g))
    for g in sp_chunks:
        if g not in emitted:
            tail.append(("relayout", g))
            emitted.add(g)

    nc = bacc.Bacc()
    zhl_d = nc.dram_tensor("zhl", [2, 2, 128, ntiles * 128], bf16, kind="ExternalInput")
    nd3_d = nc.dram_tensor("nd3", [3, N], bf16, kind="ExternalInput")
    dhl_d = nc.dram_tensor("dhl", [2, 2, 128, N], bf16, kind="ExternalInput")
    nb_d = nc.dram_tensor("nb", [128, N], f32, kind="ExternalInput")
    dic_d = nc.dram_tensor("dic", [N, D], f32, kind="ExternalInput")
    e_d = nc.dram_tensor("e", [ntiles * 128, D], f32, kind="ExternalOutput")

    npair = (ntiles + 1) // 2

    ctx = ExitStack()
    with ctx:
        def sb(name, shape, dt):
            return ctx.enter_context(nc.sbuf_tensor(name, list(shape), dt))

        zhl = [sb(f"zhl{p}", (128, 2, 2, 256), bf16) for p in range(NZBUF)]
        wu = sb("warmup", (128, 512), bf16)
        dhl = sb("dhl_sb", (128, 2, 2, N), bf16)
        nb = sb("nb_sb", (128, N), f32)
        nd3_sb = sb("nd3_sb", (3, N), bf16)
        ones3 = sb("ones3", (3, 128), bf16)
        scores = [sb(f"scores{p}", (128, N), f32) for p in range(4)]
        m8 = [sb(f"m8_{p}", (128, 8), f32) for p in range(2)]
        staging = sb("staging", (128, ntiles, 8), u16)
        idxs2 = [sb(f"idxs{q}", (128, CHUNK, 8), i16) for q in range(2)]
        gth = [sb(f"gth{q}", (128, CHUNK, D), f32) for q in range(2)]
        ps = [ctx.enter_context(nc.psum_tensor(f"ps{j}", [128, N], f32))
              for j in range(4)]

        sem = {}
        for s in ("prepd", "prepdl", "prepn", "prep3", "m1", "z", "actba",
                  "actbb", "pem", "acts", "dve", "relp", "rels", "gth",
                  "outs", "outa"):
            sem[s] = ctx.enter_context(nc.semaphore(s))

        def relayout(eng, g):
            s, e = CHUNK * g, CHUNK * (g + 1)
            q = g % 2
            rs = sem[rel_sem[g]]
            eng.wait_ge(sem["dve"], e)
            if g >= 2:
                eng.wait_ge(sem["gth"], 16 * (g - 1))
            with nc.allow_non_contiguous_dma(reason="16x2B idx relayout"):
                for kk in range(8):
                    eng.dma_start(
                        out=idxs2[q][0:16, :, kk:kk + 1],
                        in_=staging[16 * kk:16 * (kk + 1), s:e, 0:1].bitcast(i16),
                    ).then_inc(rs, 16)
            if REPLICATE:
                eng.wait_ge(rs, rel_base[g] + 16 * 8)
                for r in range(1, 8):
                    eng.dma_start(
                        out=idxs2[q][16 * r:16 * (r + 1), :, :],
                        in_=idxs2[q][0:16, :, :],
                    ).then_inc(rs, 16)

        def store(eng, g, outsem="outs"):
            eng.wait_ge(sem["gth"], 16 * (g + 1))
            eng.dma_start(
                out=e_d[CHUNK * 128 * g:CHUNK * 128 * (g + 1), :].rearrange(
                    "(c p) d -> p c d", p=128),
                in_=gth[g % 2][:],
            ).then_inc(sem[outsem], 16)

        with nc.Block() as block:

            @block.sync
            def _(sync):
                sync.dma_start(
                    out=zhl[0][:],
                    in_=zhl_d[:, :, :, 0:256].rearrange("s c p t -> p s c t"),
                ).then_inc(sem["z"], 16)
                sync.dma_start(out=dhl[:, 0, :, :],
                               in_=dhl_d[0].rearrange("c p n -> p c n")
                               ).then_inc(sem["prepd"], 16)
                sync.dma_start(out=nd3_sb[:], in_=nd3_d[:]).then_inc(sem["prep3"], 16)
                sync.dma_start(out=nb[:], in_=nb_d[:]).then_inc(sem["prepn"], 16)
                sync.dma_start(out=dhl[:, 1, :, :],
                               in_=dhl_d[1].rearrange("c p n -> p c n")
                               ).then_inc(sem["prepdl"], 16)
                for j in range(1, npair):
                    i = 2 * j
                    if i >= 2 * NZBUF:
                        sync.wait_ge(sem["pem"], 2 * (i - 2 * NZBUF) + 4)
                    sync.dma_start(
                        out=zhl[j % NZBUF][:],
                        in_=zhl_d[:, :, :, i * 128:(i + 2) * 128].rearrange(
                            "s c p t -> p s c t"),
                    ).then_inc(sem["z"], 16)
                    for g in store_pos.get(i, []) + store_pos.get(i + 1, []):
                        store(sync, g)
                for kind, v in tail:
                    if kind == "relayout":
                        relayout(sync, v)
                    else:
                        store(sync, v)
                sync.wait_ge(sem["outs"], 16 * (nchunk - len(act_stores)))
                if act_stores:
                    sync.wait_ge(sem["outa"], 16 * len(act_stores))

            @block.scalar
            def _(scalar):
                scalar.wait_ge(sem["prepn"], 16)
                for i in range(ntiles):
                    p = i % 4
                    if i >= 4:
                        scalar.wait_ge(sem["dve"], i - 3)
                    scalar.wait_ge(sem["pem"], 2 * i + 1)
                    scalar.copy(scores[p][:, 0:512], ps[p][:, 0:512])
                    if i + 4 < ntiles:
                        scalar.drain()
                        scalar.copy(ps[p][:, 0:512],
                                    nb[:, 0:512]).then_inc(sem["actba"], 1)
                    scalar.wait_ge(sem["pem"], 2 * i + 2)
                    scalar.copy(scores[p][:, 512:1024],
                                ps[p][:, 512:1024]).then_inc(sem["acts"], 1)
                    if i + 4 < ntiles:
                        scalar.drain()
                        scalar.copy(ps[p][:, 512:1024],
                                    nb[:, 512:1024]).then_inc(sem["actbb"], 1)
                for g in act_stores:
                    store(scalar, g, outsem="outa")

            @block.tensor
            def _(tensor):
                # pstate warmup: dummy matmuls on garbage while input DMAs
                # land, so the PE clock is at full speed for the real stream
                for _ in range(12):
                    tensor.matmul(ps[0][:, 0:512], wu[:, 0:128], wu[:, 0:512],
                                  start=True, stop=True, skip_group_check=True)
                tensor.wait_ge(sem["prepd"], 16)
                tensor.wait_ge(sem["prep3"], 16)
                tensor.wait_ge(sem["m1"], 1)
                for i in range(ntiles):
                    p = i % 4
                    q = (i // 2) % NZBUF
                    tsl = slice(128 * (i % 2), 128 * (i % 2) + 128)
                    tensor.wait_ge(sem["z"], 16 * (i // 2 + 1))
                    for h in range(2):
                        if i >= 4:
                            tensor.wait_ge(sem["actba" if h == 0 else "actbb"],
                                           i - 3)
                        ns = bass.ts(h, 512)
                        mm = None
                        if i < 4:
                            tensor.matmul(ps[p][:, ns], ones3[:], nd3_sb[:, ns],
                                          start=True, stop=False,
                                          skip_group_check=True)
                        for t, (sz, sd) in enumerate(TERMS):
                            if i == 0 and h == 0 and sd == 1:
                                tensor.wait_ge(sem["prepdl"], 16)
                            for c in range(2):
                                mm = tensor.matmul(ps[p][:, ns],
                                                   zhl[q][:, sz, c, tsl],
                                                   dhl[:, sd, c, ns],
                                                   start=False,
                                                   stop=(t == 2 and c == 1),
                                                   skip_group_check=True)
                        mm.then_inc(sem["pem"], 1)

            @block.vector
            def _(vector):
                vector.memset(ones3[:], 1.0)
                vector.drain()
                vector.nop().then_inc(sem["m1"], 1)
                for i in range(ntiles):
                    p = i % 4
                    vector.wait_ge(sem["acts"], i + 1)
                    vector.max(m8[i % 2][:], scores[p][:])
                    vector.drain()
                    vector.max_index(staging[:, i, :], m8[i % 2][:],
                                     scores[p][:]).then_inc(sem["dve"], 1)

            @block.gpsimd
            def _(gpsimd):
                reg = gpsimd.to_reg(CHUNK * 128)
                for g in range(nchunk):
                    if g in pool_chunks:
                        relayout(gpsimd, g)
                    gpsimd.wait_ge(sem[rel_sem[g]], rel_after[g])
                    if g >= 2:
                        gpsimd.wait_ge(sem["outs"], 16 * (g - 1))
                    if g >= 1:
                        gpsimd.wait_ge(sem["gth"], 16 * g)
                    gpsimd.dma_gather(
                        out_ap=gth[g % 2][:],
                        in_ap=dic_d[:],
                        idxs_ap=idxs2[g % 2][:],
                        num_idxs=CHUNK * 128,
                        num_idxs_reg=reg,
                        elem_size=D,
                        elem_step=D,
                    ).then_inc(sem["gth"], 16)

    nc.finalize()
    return nc


def _prep_dict(dictionary):
    dic = np.ascontiguousarray(dictionary.astype(np.float32))
    dT2 = np.ascontiguousarray(2.0 * dic.T).astype(np.float32)   # [D, N]
    dh = dT2.astype(ml_dtypes.bfloat16)
    dl = (dT2 - dh.astype(np.float32)).astype(ml_dtypes.bfloat16)
    dhl = np.ascontiguousarray(
        np.stack([dh.reshape(2, 128, N), dl.reshape(2, 128, N)]))
    nd = -(dic.astype(np.float64) ** 2).sum(-1)
    nb = np.ascontiguousarray(
        np.broadcast_to(nd.astype(np.float32), (128, N)))
    h1 = nd.astype(ml_dtypes.bfloat16)
    r1 = nd - h1.astype(np.float64)
    h2 = r1.astype(ml_dtypes.bfloat16)
    r2 = r1 - h2.astype(np.float64)
    h3 = r2.astype(ml_dtypes.bfloat16)
    nd3 = np.ascontiguousarray(np.stack([h1, h2, h3]).astype(ml_dtypes.bfloat16))
    return dic, dhl, nb, nd3


def _prep_ze(ze_core):
    zh = ze_core.astype(ml_dtypes.bfloat16)
    zl = (ze_core - zh.astype(np.float32)).astype(ml_dtypes.bfloat16)
    rows = ze_core.shape[0]
    return np.ascontiguousarray(
        np.stack([np.ascontiguousarray(zh.T).reshape(2, 128, rows),
                  np.ascontiguousarray(zl.T).reshape(2, 128, rows)]))


def kernel(ze, dictionary):
    if "nc" not in _CACHE:
        _CACHE["nc"] = build()
    nc = _CACHE["nc"]
    dic, dhl, nb, nd3 = _prep_dict(np.asarray(dictionary))
    ze = np.ascontiguousarray(np.asarray(ze, dtype=np.float32))
    zec = ze.reshape(CORES, ROWS, D)
    in_maps = [{"zhl": _prep_ze(zec[c]), "dhl": dhl, "nb": nb, "nd3": nd3,
                "dic": dic} for c in range(CORES)]
    res = run_bass_kernel_spmd(nc, in_maps, list(range(CORES)))
    e = np.stack([res.results[c]["e"] for c in range(CORES)])
    return e.reshape(B, T, D)


# revision 30
# speedup vs baseline: 2.3020x; 1.0076x over previous
"""VQ codebook-lookup kernel for Trainium2 (8 NeuronCores, data-parallel over batch).

e[b,t,:] = dictionary[argmin_n ||ze[b,t,:] - dictionary[n,:]||^2]

Per core: 4 batches x 2048 rows = 8192, tiled 64 x 128 rows.
score(t,n) = 2*ze.c_n - |c_n|^2; argmax_n score == argmin_n d2.

Host splits ze and 2*dict^T into bf16 hi/lo pairs (pre-transposed), giving
~18-bit operands; scores = zh.dh + zh.dl + zl.dh accumulated in fp32 PSUM on
top of an Act-preloaded -|c|^2 bias row (matmuls run start=False). No PE
transposes, no norm matmul: 12 bf16 MMs per 128-row tile is the whole PE load.
Act copies scores PSUM->SBUF; DVE max + max_index give the argmax. Index
relayout to the 16-partition gather format runs on Pool SWDGE for the early
batches (keeps SP's HWDGE path free for loads) and on SP for the tail; Pool
dma_gather fetches the codebook rows.
"""
import sys
if '/opt/trn_rl_repo' not in sys.path:
    sys.path.insert(0, '/opt/trn_rl_repo')

import numpy as np
import ml_dtypes
from contextlib import ExitStack

import concourse.bass as bass
import concourse.bacc as bacc
import concourse.mybir as mybir
from concourse.bass_utils import run_bass_kernel_spmd

B, T, D, N = 32, 2048, 256, 1024
CORES = 8
ROWS = (B // CORES) * T          # 8192 rows per core
NTILES = ROWS // 128             # 64
CHUNK = 8                        # tiles per gather chunk
REPLICATE = True
NZBUF = 8                        # zhl pair-load prefetch depth (16 tiles)

f32 = mybir.dt.float32
bf16 = mybir.dt.bfloat16
u16 = mybir.dt.uint16
i16 = mybir.dt.int16

_CACHE = {}

TERMS = [(0, 0), (1, 0), (0, 1)]  # (z plane, d plane): hh, lh, hl — dl last
                                  # so the split dict-lo load can arrive late


def build(ntiles=NTILES):
    nchunk = ntiles // CHUNK
    ndma_b = 8 + (7 if REPLICATE else 0)
    # per-chunk relayout into offset-0 ping-pong buffers (the only layout the
    # hw gather path tolerates). Chunks 0..nchunk-2 relayout on Pool SWDGE;
    # the final chunk's relayout runs on SP HWDGE at the tail.
    pool_chunks = list(range(nchunk - 1))
    sp_chunks = [nchunk - 1] if nchunk else []
    rel_sem = {}
    rel_after = {}
    rel_base = {}
    for group, name in ((pool_chunks, "relp"), (sp_chunks, "rels")):
        for j, g in enumerate(group):
            rel_sem[g] = name
            rel_base[g] = 16 * ndma_b * j
            rel_after[g] = 16 * ndma_b * (j + 1)
    store_pos = {}
    inline_stores = set()
    for g in range(nchunk):
        pos = CHUNK * g + 34
        if pos < ntiles:
            store_pos.setdefault(pos, []).append(g)
            inline_stores.add(g)
    # SP tail: leftover stores ascending; the final chunk's relayout is
    # emitted one store earlier than strictly needed (it only gates on dve,
    # which resolves before the preceding store's gather does)
    tail_stores = [g for g in range(nchunk) if g not in inline_stores]
    act_stores = tail_stores[-2:] if len(tail_stores) >= 2 else []
    tail = []
    emitted = set(pool_chunks)
    for g in tail_stores:
        if g in act_stores:
            continue
        tail.append(("store", g))
    for g in sp_chunks:
        if g not in emitted:
            tail.append(("relayout", g))
            emitted.add(g)

    nc = bacc.Bacc()
    zhl_d = nc.dram_tensor("zhl", [2, 2, 128, ntiles * 128], bf16, kind="ExternalInput")
    nd3_d = nc.dram_tensor("nd3", [3, N], bf16, kind="ExternalInput")
    dhl_d = nc.dram_tensor("dhl", [2, 2, 128, N], bf16, kind="ExternalInput")
    nb_d = nc.dram_tensor("nb", [128, N], f32, kind="ExternalInput")
    dic_d = nc.dram_tensor("dic", [N, D], f32, kind="ExternalInput")
    e_d = nc.dram_tensor("e", [ntiles * 128, D], f32, kind="ExternalOutput")

    npair = (ntiles + 1) // 2

    ctx = ExitStack()
    with ctx:
        def sb(name, shape, dt):
            return ctx.enter_context(nc.sbuf_tensor(name, list(shape), dt))

        zhl = [sb(f"zhl{p}", (128, 2, 2, 256), bf16) for p in range(NZBUF)]
        wu = sb("warmup", (128, 512), bf16)
        dhl = sb("dhl_sb", (128, 2, 2, N), bf16)
        nb = sb("nb_sb", (128, N), f32)
        nd3_sb = sb("nd3_sb", (3, N), bf16)
        ones3 = sb("ones3", (3, 128), bf16)
        scores = [sb(f"scores{p}", (128, N), f32) for p in range(4)]
        m8 = [sb(f"m8_{p}", (128, 8), f32) for p in range(2)]
        staging = sb("staging", (128, ntiles, 8), u16)
        idxs2 = [sb(f"idxs{q}", (128, CHUNK, 8), i16) for q in range(2)]
        gth = [sb(f"gth{q}", (128, CHUNK, D), f32) for q in range(2)]
        ps = [ctx.enter_context(nc.psum_tensor(f"ps{j}", [128, N], f32))
              for j in range(4)]

        sem = {}
        for s in ("prepd", "prepdl", "prepn", "prep3", "m1", "z", "actba",
                  "actbb", "pem", "acts", "dve", "relp", "rels", "gth",
                  "outs", "outa"):
            sem[s] = ctx.enter_context(nc.semaphore(s))

        def relayout(eng, g):
            s, e = CHUNK * g, CHUNK * (g + 1)
            q = g % 2
            rs = sem[rel_sem[g]]
            eng.wait_ge(sem["dve"], e)
            if g >= 2:
                eng.wait_ge(sem["gth"], 16 * (g - 1))
            with nc.allow_non_contiguous_dma(reason="16x2B idx relayout"):
                for kk in range(8):
                    eng.dma_start(
                        out=idxs2[q][0:16, :, kk:kk + 1],
                        in_=staging[16 * kk:16 * (kk + 1), s:e, 0:1].bitcast(i16),
                    ).then_inc(rs, 16)
            if REPLICATE:
                eng.wait_ge(rs, rel_base[g] + 16 * 8)
                for r in range(1, 8):
                    eng.dma_start(
                        out=idxs2[q][16 * r:16 * (r + 1), :, :],
                        in_=idxs2[q][0:16, :, :],
                    ).then_inc(rs, 16)

        def store(eng, g, outsem="outs"):
            eng.wait_ge(sem["gth"], 16 * (g + 1))
            eng.dma_start(
                out=e_d[CHUNK * 128 * g:CHUNK * 128 * (g + 1), :].rearrange(
                    "(c p) d -> p c d", p=128),
                in_=gth[g % 2][:],
            ).then_inc(sem[outsem], 16)

        with nc.Block() as block:

            @block.sync
            def _(sync):
                sync.dma_start(
                    out=zhl[0][:],
                    in_=zhl_d[:, :, :, 0:256].rearrange("s c p t -> p s c t"),
                ).then_inc(sem["z"], 16)
                sync.dma_start(out=dhl[:, 0, :, :],
                               in_=dhl_d[0].rearrange("c p n -> p c n")
                               ).then_inc(sem["prepd"], 16)
                sync.dma_start(out=nd3_sb[:], in_=nd3_d[:]).then_inc(sem["prep3"], 16)
                sync.dma_start(out=nb[:], in_=nb_d[:]).then_inc(sem["prepn"], 16)
                sync.dma_start(out=dhl[:, 1, :, :],
                               in_=dhl_d[1].rearrange("c p n -> p c n")
                               ).then_inc(sem["prepdl"], 16)
                for j in range(1, npair):
                    i = 2 * j
                    if i >= 2 * NZBUF:
                        sync.wait_ge(sem["pem"], 2 * (i - 2 * NZBUF) + 4)
                    sync.dma_start(
                        out=zhl[j % NZBUF][:],
                        in_=zhl_d[:, :, :, i * 128:(i + 2) * 128].rearrange(
                            "s c p t -> p s c t"),
                    ).then_inc(sem["z"], 16)
                    for g in store_pos.get(i, []) + store_pos.get(i + 1, []):
                        store(sync, g)
                for kind, v in tail:
                    if kind == "relayout":
                        relayout(sync, v)
                    else:
                        store(sync, v)
                sync.wait_ge(sem["outs"], 16 * (nchunk - len(act_stores)))
                if act_stores:
                    n_outa = sum(2 if g == nchunk - 1 else 1 for g in act_stores)
                    sync.wait_ge(sem["outa"], 16 * n_outa)

            @block.scalar
            def _(scalar):
                scalar.wait_ge(sem["prepn"], 16)
                for i in range(ntiles):
                    p = i % 4
                    if i >= 4:
                        scalar.wait_ge(sem["dve"], i - 3)
                    scalar.wait_ge(sem["pem"], 2 * i + 1)
                    scalar.copy(scores[p][:, 0:512], ps[p][:, 0:512])
                    if i + 4 < ntiles:
                        scalar.drain()
                        scalar.copy(ps[p][:, 0:512],
                                    nb[:, 0:512]).then_inc(sem["actba"], 1)
                    scalar.wait_ge(sem["pem"], 2 * i + 2)
                    scalar.copy(scores[p][:, 512:1024],
                                ps[p][:, 512:1024]).then_inc(sem["acts"], 1)
                    if i + 4 < ntiles:
                        scalar.drain()
                        scalar.copy(ps[p][:, 512:1024],
                                    nb[:, 512:1024]).then_inc(sem["actbb"], 1)
                for g in act_stores:
                    if g == nchunk - 1:
                        for half in range(2):
                            scalar.wait_ge(sem["gth"], 16 * (g + 1 + half))
                            r0 = CHUNK * 128 * g + 512 * half
                            scalar.dma_start(
                                out=e_d[r0:r0 + 512, :].rearrange(
                                    "(c p) d -> p c d", p=128),
                                in_=gth[g % 2][:, 4 * half:4 * half + 4, :],
                            ).then_inc(sem["outa"], 16)
                    else:
                        store(scalar, g, outsem="outa")

            @block.tensor
            def _(tensor):
                # pstate warmup: dummy matmuls on garbage while input DMAs
                # land, so the PE clock is at full speed for the real stream
                for _ in range(12):
                    tensor.matmul(ps[0][:, 0:512], wu[:, 0:128], wu[:, 0:512],
                                  start=True, stop=True, skip_group_check=True)
                tensor.wait_ge(sem["prepd"], 16)
                tensor.wait_ge(sem["prep3"], 16)
                tensor.wait_ge(sem["m1"], 1)
                for i in range(ntiles):
                    p = i % 4
                    q = (i // 2) % NZBUF
                    tsl = slice(128 * (i % 2), 128 * (i % 2) + 128)
                    tensor.wait_ge(sem["z"], 16 * (i // 2 + 1))
                    for h in range(2):
                        if i >= 4:
                            tensor.wait_ge(sem["actba" if h == 0 else "actbb"],
                                           i - 3)
                        ns = bass.ts(h, 512)
                        mm = None
                        if i < 4:
                            tensor.matmul(ps[p][:, ns], ones3[:], nd3_sb[:, ns],
                                          start=True, stop=False,
                                          skip_group_check=True)
                        for t, (sz, sd) in enumerate(TERMS):
                            if i == 0 and h == 0 and sd == 1:
                                tensor.wait_ge(sem["prepdl"], 16)
                            for c in range(2):
                                mm = tensor.matmul(ps[p][:, ns],
                                                   zhl[q][:, sz, c, tsl],
                                                   dhl[:, sd, c, ns],
                                                   start=False,
                                                   stop=(t == 2 and c == 1),
                                                   skip_group_check=True)
                        mm.then_inc(sem["pem"], 1)

            @block.vector
            def _(vector):
                vector.memset(ones3[:], 1.0)
                vector.drain()
                vector.nop().then_inc(sem["m1"], 1)
                for i in range(ntiles):
                    p = i % 4
                    vector.wait_ge(sem["acts"], i + 1)
                    vector.max(m8[i % 2][:], scores[p][:])
                    vector.drain()
                    vector.max_index(staging[:, i, :], m8[i % 2][:],
                                     scores[p][:]).then_inc(sem["dve"], 1)

            @block.gpsimd
            def _(gpsimd):
                reg = gpsimd.to_reg(CHUNK * 128)
                for g in range(nchunk):
                    if g in pool_chunks:
                        relayout(gpsimd, g)
                    if g != nchunk - 1:
                        gpsimd.wait_ge(sem[rel_sem[g]], rel_after[g])
                    if g >= 2:
                        gpsimd.wait_ge(sem["outs"], 16 * (g - 1))
                    if g >= 1:
                        gpsimd.wait_ge(sem["gth"], 16 * g)
                    if g == nchunk - 1:
                        gpsimd.wait_ge(sem[rel_sem[g]], rel_after[g])
                        reg2 = gpsimd.to_reg(CHUNK * 64)
                        for half in range(2):
                            gpsimd.dma_gather(
                                out_ap=gth[g % 2][:, 4 * half:4 * half + 4, :],
                                in_ap=dic_d[:],
                                idxs_ap=idxs2[g % 2][:, 4 * half:4 * half + 4, :],
                                num_idxs=CHUNK * 64,
                                num_idxs_reg=reg2,
                                elem_size=D,
                                elem_step=D,
                            ).then_inc(sem["gth"], 16)
                    else:
                        gpsimd.dma_gather(
                            out_ap=gth[g % 2][:],
                            in_ap=dic_d[:],
                            idxs_ap=idxs2[g % 2][:],
                            num_idxs=CHUNK * 128,
                            num_idxs_reg=reg,
                            elem_size=D,
                            elem_step=D,
                        ).then_inc(sem["gth"], 16)

    nc.finalize()
    return nc


def _prep_dict(dictionary):
    dic = np.ascontiguousarray(dictionary.astype(np.float32))
    dT2 = np.ascontiguousarray(2.0 * dic.T).astype(np.float32)   # [D, N]
    dh = dT2.astype(ml_dtypes.bfloat16)
    dl = (dT2 - dh.astype(np.float32)).astype(ml_dtypes.bfloat16)
    dhl = np.ascontiguousarray(
        np.stack([dh.reshape(2, 128, N), dl.reshape(2, 128, N)]))
    nd = -(dic.astype(np.float64) ** 2).sum(-1)
    nb = np.ascontiguousarray(
        np.broadcast_to(nd.astype(np.float32), (128, N)))
    h1 = nd.astype(ml_dtypes.bfloat16)
    r1 = nd - h1.astype(np.float64)
    h2 = r1.astype(ml_dtypes.bfloat16)
    r2 = r1 - h2.astype(np.float64)
    h3 = r2.astype(ml_dtypes.bfloat16)
    nd3 = np.ascontiguousarray(np.stack([h1, h2, h3]).astype(ml_dtypes.bfloat16))
    return dic, dhl, nb, nd3


def _prep_ze(ze_core):
    zh = ze_core.astype(ml_dtypes.bfloat16)
    zl = (ze_core - zh.astype(np.float32)).astype(ml_dtypes.bfloat16)
    rows = ze_core.shape[0]
    return np.ascontiguousarray(
        np.stack([np.ascontiguousarray(zh.T).reshape(2, 128, rows),
                  np.ascontiguousarray(zl.T).reshape(2, 128, rows)]))


def kernel(ze, dictionary):
    if "nc" not in _CACHE:
        _CACHE["nc"] = build()
    nc = _CACHE["nc"]
    dic, dhl, nb, nd3 = _prep_dict(np.asarray(dictionary))
    ze = np.ascontiguousarray(np.asarray(ze, dtype=np.float32))
    zec = ze.reshape(CORES, ROWS, D)
    in_maps = [{"zhl": _prep_ze(zec[c]), "dhl": dhl, "nb": nb, "nd3": nd3,
                "dic": dic} for c in range(CORES)]
    res = run_bass_kernel_spmd(nc, in_maps, list(range(CORES)))
    e = np.stack([res.results[c]["e"] for c in range(CORES)])
    return e.reshape(B, T, D)
